# revision 9
# baseline (speedup 1.0000x reference)
"""MoE layer (top-2 of 24 experts, d_model=1024, d_ff=4096, T=4096 tokens)
on 8 Trainium2 NeuronCores.

Strategy (expert-parallel, host-routed):
  - Host computes the gate, top-2 ids and softmax probs, gathers each
    expert's tokens into a transposed buffer xT[e] = [D, C_j].
  - Experts sharded 3 per core, balanced by token count (sorted-deal into
    3 slots); slot capacity = max count in that octile, 16-aligned (exact,
    NOT padded to 128 -- phase A matmul N is the exact token count).
  - Per expert on device:
      phase A: ht[m] = gelu(w1_km.T @ xT + b1)   32 m-groups, N=C tokens
      phase B: k-outer sweep, PSUM-resident accumulators pb[n][s]
               (2*ceil(C/128) <= 8 banks) so w2 streams HBM exactly once:
               pb[n][s] += ht[k][:,s].T-chunk @ w2_k[:, n*512:...]
      drain:   y[n][s] = prob * pb[n][s]  (DVE), DMA out
  - Weight DMAs (sync ring) are emitted in deadline-sorted order so w2[e]
    prefetches during A_e and w1[e+1] interleaves behind it; pool buffer
    recycling provides the runtime pacing.
  - Host scatters the two per-token expert outputs back together.

Matmuls in bf16 with fp32 PSUM accumulation (rel err ~3e-3); b1 applied
exactly as the ACT per-partition bias.
"""

import numpy as np
import ml_dtypes

P = 128
D_MODEL = 1024
D_FF = 4096
NUM_EXPERTS = 24
TOP_K = 2
N_CORES = 8
E_LOC = NUM_EXPERTS // N_CORES   # 3 experts per core
KD = D_MODEL // P                # 8  k-chunks over d_model
KF = D_FF // P                   # 32 k-chunks over d_ff
ND = D_MODEL // 512              # 2  512-wide output chunks
W1C = 4                          # w1 column chunks per k-tile
W1CW = D_FF // W1C               # 1024 columns per chunk
MPC = W1CW // P                  # 8 m-tiles per chunk
BF16 = ml_dtypes.bfloat16


def _stiles(C):
    """128-token sub-tiles of a slot: [(offset, rows)]."""
    return [(s * P, min(P, C - s * P)) for s in range((C + P - 1) // P)]


def _build(Cs, repeat=1):
    """Per-core Bass program (SPMD: same program, per-core data).

    Cs: per-slot token capacities (16-aligned, each <= 512).
    """
    import concourse.bacc as bacc
    import concourse.mybir as mybir
    from concourse.tile import TileContext

    dt = mybir.dt.bfloat16
    f32 = mybir.dt.float32
    CT = sum(Cs)
    offs = [sum(Cs[:j]) for j in range(E_LOC)]
    nsj = [len(_stiles(C)) for C in Cs]
    scol = [sum(nsj[:j]) for j in range(E_LOC)]
    NS = sum(nsj)

    # model timeline (us) for DMA deadline sorting
    tA = [256.0 * C / 2400.0 for C in Cs]
    tB = [2.0 * ns * KF * 512.0 / 2400.0 for ns in nsj]
    t0 = []
    t = 0.0
    for j in range(E_LOC):
        t0.append(t)
        t += tA[j] + tB[j]

    events = []  # (deadline, kind, e, k, q)
    for e in range(E_LOC):
        for q in range(W1C):
            for k in range(KD):
                events.append((t0[e] + tA[e] * q / W1C - 12.0, 0, e, k, q))
        for k in range(KF):
            events.append((t0[e] + tA[e] + tB[e] * k / KF - 6.0, 1, e, k, 0))
    events.sort(key=lambda ev: (ev[0], ev[1], ev[2], ev[4], ev[3]))

    nc = bacc.Bacc(None, target_bir_lowering=False)
    xt_d = [nc.dram_tensor(f"xt{j}", [KD, P, Cs[j]], dt, kind="ExternalInput")
            for j in range(E_LOC)]
    w1 = nc.dram_tensor("w1", [E_LOC, KD, W1C, P, W1CW], dt,
                        kind="ExternalInput")
    w2 = nc.dram_tensor("w2", [E_LOC, KF, P, D_MODEL], dt,
                        kind="ExternalInput")
    pr = nc.dram_tensor("pr", [P, NS], f32, kind="ExternalInput")
    b1 = nc.dram_tensor("b1", [P, E_LOC * KF], f32, kind="ExternalInput")
    y = nc.dram_tensor("y", [ND, CT, 512], f32, kind="ExternalOutput")

    with TileContext(nc) as tc:
        with tc.tile_pool(name="consts", bufs=1) as consts, \
             tc.tile_pool(name="xtp", bufs=E_LOC * KD) as xtp, \
             tc.tile_pool(name="w1p", bufs=40) as w1p, \
             tc.tile_pool(name="w2p", bufs=16) as w2p, \
             tc.tile_pool(name="htp", bufs=KF + 2) as htp, \
             tc.tile_pool(name="outp", bufs=4) as outp, \
             tc.tile_pool(name="psp", bufs=8, space="PSUM") as psp:

            b1_t = consts.tile([P, E_LOC * KF], f32, tag="b1")
            nc.scalar.dma_start(b1_t[:], b1[:, :])
            pr_t = consts.tile([P, NS], f32, tag="pr")
            nc.scalar.dma_start(pr_t[:], pr[:, :])

            xts = {}
            for j in range(E_LOC):
                for k in range(KD):
                    t_ = xtp.tile([P, Cs[j]], dt, tag=f"xt{j}")
                    nc.scalar.dma_start(t_[:], xt_d[j][k, :, :])
                    xts[(j, k)] = t_

            w1ts = {}
            w2ts = {}
            cur = [0]

            def emit_until(tnow):
                while cur[0] < len(events) and events[cur[0]][0] <= tnow:
                    _, kind, e, k, q = events[cur[0]]
                    cur[0] += 1
                    if kind == 0:
                        t_ = w1p.tile([P, W1CW], dt, tag="w1")
                        nc.sync.dma_start(t_[:], w1[e, k, q, :, :])
                        w1ts[(e, k, q)] = t_
                    else:
                        t_ = w2p.tile([P, D_MODEL], dt, tag="w2")
                        nc.sync.dma_start(t_[:], w2[e, k, :, :])
                        w2ts[(e, k)] = t_

            mt = 0.0
            for _ in range(repeat):
                for e in range(E_LOC):
                    C = Cs[e]
                    sts = _stiles(C)
                    ns = len(sts)
                    # phase A: ht[m] = gelu(w1.T @ x + b1), [P dff x C tok]
                    hts = []
                    for m in range(KF):
                        emit_until(mt)
                        pa = psp.tile([P, 512], f32, tag="ps")
                        for k in range(KD):
                            nc.tensor.matmul(
                                pa[:, :C],
                                w1ts[(e, k, m // MPC)][:, (m % MPC) * P:
                                                       (m % MPC + 1) * P],
                                xts[(e, k)][:, :],
                                start=(k == 0), stop=(k == KD - 1))
                        ht = htp.tile([P, Cs[0]], dt, tag="ht")
                        nc.scalar.activation(
                            ht[:, :C], pa[:, :C],
                            mybir.ActivationFunctionType.Gelu,
                            bias=b1_t[:, e * KF + m: e * KF + m + 1])
                        hts.append(ht)
                        mt += tA[e] / KF
                    # phase B: k-outer, accumulators live in PSUM
                    pbs = [[psp.tile([P, 512], f32, tag="ps", name="pb")
                            for _s in range(ns)] for _n in range(ND)]
                    for k in range(KF):
                        emit_until(mt)
                        for s, (soff, M) in enumerate(sts):
                            for n in range(ND):
                                nc.tensor.matmul(
                                    pbs[n][s][:M, :],
                                    hts[k][:, soff:soff + M],
                                    w2ts[(e, k)][:, n * 512:(n + 1) * 512],
                                    start=(k == 0), stop=(k == KF - 1))
                        mt += tB[e] / KF
                    # drain: y = prob * pb
                    for n in range(ND):
                        for s, (soff, M) in enumerate(sts):
                            ot = outp.tile([P, 512], f32, tag="out")
                            nc.vector.tensor_scalar_mul(
                                ot[:M, :], pbs[n][s][:M, :],
                                pr_t[:M, scol[e] + s: scol[e] + s + 1])
                            nc.scalar.dma_start(
                                y[n, offs[e] + soff: offs[e] + soff + M, :],
                                ot[:M, :])
    nc.finalize()
    return nc


def _route(x, gate_w, gate_b):
    """Top-2 routing on host. Returns flattened (expert, prob) per routed
    pair, the by-expert sort order, per-expert counts/starts, and each
    pair's position within its expert segment."""
    T = x.shape[0]
    scores = x @ gate_w + gate_b                      # [T, E]
    part = np.argpartition(scores, -TOP_K, axis=1)[:, -TOP_K:]   # [T, 2]
    vals = np.take_along_axis(scores, part, axis=1)
    vmax = vals.max(axis=1, keepdims=True)
    ex = np.exp(vals - vmax)
    prob = ex / ex.sum(axis=1, keepdims=True)

    expert_flat = part.ravel()                        # [2T]
    prob_flat = prob.ravel().astype(np.float32)
    token_flat = np.repeat(np.arange(T), TOP_K)

    order = np.argsort(expert_flat, kind="stable")
    counts = np.bincount(expert_flat, minlength=NUM_EXPERTS)
    starts = np.zeros(NUM_EXPERTS + 1, dtype=np.int64)
    np.cumsum(counts, out=starts[1:])

    inv_order = np.empty_like(order)
    inv_order[order] = np.arange(order.size)
    pos = inv_order - starts[expert_flat]
    return (expert_flat, prob_flat, token_flat, order, counts, starts, pos)


def _prepare(x, gate_w, gate_b, w1, b1, w2, b2):
    """Host-side routing, balanced expert->(core,slot) assignment, and
    per-core input packing. Returns (in_maps, Cs, meta-for-combine)."""
    B, S, D = x.shape
    T = B * S
    xf = np.ascontiguousarray(x.reshape(T, D), dtype=np.float32)

    (expert_flat, prob_flat, token_flat, order, counts, starts, pos) = _route(
        xf, np.asarray(gate_w, np.float32), np.asarray(gate_b, np.float32))

    # balanced assignment: slot j of core c holds expert_desc[j*8 + c]
    expert_desc = np.argsort(-counts, kind="stable")
    core_of = np.empty(NUM_EXPERTS, dtype=np.int64)
    slot_of = np.empty(NUM_EXPERTS, dtype=np.int64)
    for j in range(E_LOC):
        for c in range(N_CORES):
            e = expert_desc[j * N_CORES + c]
            core_of[e] = c
            slot_of[e] = j
    Cs = []
    for j in range(E_LOC):
        mx = int(counts[expert_desc[j * N_CORES:(j + 1) * N_CORES]].max())
        Cs.append(max(16, -(-mx // 16) * 16))        # 16-aligned exact cap
        assert Cs[j] <= 512
    CT = sum(Cs)
    offs = [sum(Cs[:j]) for j in range(E_LOC)]
    nsj = [len(_stiles(C)) for C in Cs]
    scol = [sum(nsj[:j]) for j in range(E_LOC)]
    NS = sum(nsj)

    xg16 = xf[token_flat[order]].astype(BF16)         # [2T, D] sorted by expert
    sorted_probs = prob_flat[order]

    w1_16 = np.asarray(w1, np.float32).astype(BF16)   # [E, D, F]
    w2_16 = np.asarray(w2, np.float32).astype(BF16)   # [E, F, D]
    b1_f = np.asarray(b1, np.float32)                 # [E, F]

    in_maps = []
    for c in range(N_CORES):
        m = {}
        w1_core = np.empty((E_LOC, KD, W1C, P, W1CW), dtype=BF16)
        w2_core = np.empty((E_LOC, KF, P, D_MODEL), dtype=BF16)
        b1_core = np.empty((E_LOC, D_FF), dtype=np.float32)
        pr_core = np.zeros((P, NS), dtype=np.float32)
        for j in range(E_LOC):
            e = expert_desc[j * N_CORES + c]
            c_e = int(counts[e])
            xt_j = np.zeros((D, Cs[j]), dtype=BF16)
            if c_e:
                seg = slice(starts[e], starts[e] + c_e)
                xt_j[:, :c_e] = xg16[seg].T
                tmp = np.zeros((nsj[j] * P,), dtype=np.float32)
                tmp[:c_e] = sorted_probs[seg]
                pr_core[:, scol[j]:scol[j] + nsj[j]] = \
                    tmp.reshape(nsj[j], P).T
            m[f"xt{j}"] = np.ascontiguousarray(xt_j.reshape(KD, P, Cs[j]))
            w1_core[j] = (w1_16[e].reshape(KD, P, W1C, W1CW)
                          .transpose(0, 2, 1, 3))
            w2_core[j] = w2_16[e].reshape(KF, P, D_MODEL)
            b1_core[j] = b1_f[e]
        m["w1"] = np.ascontiguousarray(w1_core)
        m["w2"] = np.ascontiguousarray(w2_core)
        m["pr"] = np.ascontiguousarray(pr_core)
        m["b1"] = np.ascontiguousarray(
            b1_core.reshape(E_LOC, KF, P).transpose(2, 0, 1)
            .reshape(P, E_LOC * KF))
        in_maps.append(m)

    meta = dict(T=T, shape=x.shape, CT=CT, offs=offs,
                core_of=core_of, slot_of=slot_of,
                expert_flat=expert_flat, prob_flat=prob_flat,
                token_flat=token_flat, pos=pos, b2=np.asarray(b2, np.float32))
    return in_maps, Cs, meta


def _combine(y_per_core, meta):
    """out[t] = sum of the token's two routed expert outputs (+ b2 term)."""
    T = meta["T"]
    CT = meta["CT"]
    offs = np.asarray(meta["offs"], dtype=np.int64)
    expert_flat = meta["expert_flat"]
    # each per-core y is [ND, CT, 512] -> [CT, 1024]
    y_flat = np.concatenate(
        [np.concatenate([yc[0], yc[1]], axis=1) for yc in y_per_core], axis=0)

    rows = (meta["core_of"][expert_flat] * CT
            + offs[meta["slot_of"][expert_flat]] + meta["pos"])
    rows = rows.reshape(T, TOP_K)
    out = y_flat[rows[:, 0]] + y_flat[rows[:, 1]]

    b2_f = meta["b2"]
    if np.any(b2_f):
        combine = np.zeros((T, NUM_EXPERTS), dtype=np.float32)
        np.add.at(combine, (meta["token_flat"], expert_flat), meta["prob_flat"])
        out += combine @ b2_f
    return np.ascontiguousarray(out.reshape(meta["shape"]), dtype=np.float32)


def kernel(x, gate_w, gate_b, w1, b1, w2, b2):
    from concourse import bass_utils

    in_maps, Cs, meta = _prepare(x, gate_w, gate_b, w1, b1, w2, b2)
    nc = _build(Cs)
    res = bass_utils.run_bass_kernel_spmd(nc, in_maps, core_ids=list(range(N_CORES)))
    return _combine([res.results[c]["y"] for c in range(N_CORES)], meta)


# revision 11
# speedup vs baseline: 1.2617x; 1.2617x over previous
"""MoE layer (top-2 of 24 experts, d_model=1024, d_ff=4096, T=4096 tokens)
on 8 Trainium2 NeuronCores.

Strategy (expert-parallel, host-routed):
  - Host computes the gate, top-2 ids and softmax probs, gathers each
    expert's tokens into a transposed, k-packed buffer per slot.
  - Experts sharded 3 per core, balanced by token count (sorted-deal into
    3 slots); slot capacity = max count in that octile, 16-aligned exact.
  - Per expert on device (all matmul free dims = exact token count C):
      phase A: ht[m] = gelu(w1_km.T @ xT + b1)    32 m-groups, N=C
      phase B (transposed): 8 PSUM-resident banks pb[md] = yT d-chunks,
               k-outer sweep so w2 streams HBM exactly once:
               pb[md] += w2_k[:, md].T @ ht[k]    N=C, no 128-padding
      drain:   yT[md] = pb[md] * prob_broadcast   (DVE), DMA out
  - DMAs are k-packed into 0.5-2 MB transfers (deep intra-DMA pipelining)
    and emitted in deadline-sorted order on two HWDGE rings (sync: w1,
    scalar: x/w2/y); pool buffer recycling provides runtime pacing.
  - ~28 warmup matmuls on a zeroed scratch tile keep the PE HAM clock
    un-throttled (K=8/8) through the startup DMA window.
  - Host scatters the two per-token expert outputs back together.

Matmuls in bf16 with fp32 PSUM accumulation (rel err ~3e-3); b1 applied
exactly as the ACT per-partition bias.
"""

import numpy as np
import ml_dtypes

P = 128
D_MODEL = 1024
D_FF = 4096
NUM_EXPERTS = 24
TOP_K = 2
N_CORES = 8
E_LOC = NUM_EXPERTS // N_CORES   # 3 experts per core
KD = D_MODEL // P                # 8  k-chunks over d_model
KF = D_FF // P                   # 32 k-chunks over d_ff
MD = D_MODEL // P                # 8  output d-chunks (phase B)
G = 4                            # w2 k-tiles packed per DMA
BF16 = ml_dtypes.bfloat16
NWARM = 28                       # PE warmup matmuls


def _w1_chunks(e):
    """(col_start, width) chunks of w1's 4096 columns for expert slot e.
    First-processed expert uses small leading chunks for a fast start."""
    if e == 0:
        return [(0, 256), (256, 256), (512, 256), (768, 256),
                (1024, 1024), (2048, 1024), (3072, 1024)]
    return [(0, 1024), (1024, 1024), (2048, 1024), (3072, 1024)]


def _build(Cs, repeat=1):
    """Per-core Bass program (SPMD: same program, per-core data).

    Cs: per-slot token capacities (16-aligned, each <= 512).
    """
    import concourse.bacc as bacc
    import concourse.mybir as mybir
    from concourse.tile import TileContext

    dt = mybir.dt.bfloat16
    f32 = mybir.dt.float32
    mult = mybir.AluOpType.mult
    CT = sum(Cs)
    offs = [sum(Cs[:j]) for j in range(E_LOC)]

    # model timeline (us) for DMA deadline sorting
    tA = [256.0 * C / 2400.0 for C in Cs]
    t0 = []
    t = 0.0
    for j in range(E_LOC):
        t0.append(t)
        t += 2.0 * tA[j]

    # (deadline, ring, kind, e, idx)  ring 0=sync(w1) 1=scalar(w2/xt/prb)
    events = []
    for e in range(E_LOC):
        for ci, (cs, w) in enumerate(_w1_chunks(e)):
            events.append((t0[e] + tA[e] * cs / D_FF - 12.0, 0, 'w1', e, ci))
        for kc in range(KF // G):
            events.append((t0[e] + tA[e] * (1.0 + kc * G / KF) - 8.0,
                           1, 'w2', e, kc))
        if e > 0:
            events.append((t0[e] - 20.0, 1, 'xt', e, 0))
    events.append((t0[0] + tA[0] + 5.0, 1, 'prb', 0, 0))
    events.sort(key=lambda ev: (ev[0], ev[1]))

    nc = bacc.Bacc(None, target_bir_lowering=False)
    xt_d = [nc.dram_tensor(f"xt{j}", [P, KD * Cs[j]], dt, kind="ExternalInput")
            for j in range(E_LOC)]
    w1 = nc.dram_tensor("w1", [E_LOC, P, KD * D_FF], dt, kind="ExternalInput")
    w2 = nc.dram_tensor("w2", [E_LOC, KF // G, P, G * D_MODEL], dt,
                        kind="ExternalInput")
    prb = nc.dram_tensor("prb", [P, CT], f32, kind="ExternalInput")
    b1 = nc.dram_tensor("b1", [P, E_LOC * KF], f32, kind="ExternalInput")
    y_d = [nc.dram_tensor(f"y{j}", [D_MODEL, Cs[j]], dt,
                          kind="ExternalOutput") for j in range(E_LOC)]

    with TileContext(nc) as tc:
        with tc.tile_pool(name="consts", bufs=1) as consts, \
             tc.tile_pool(name="w1ps", bufs=4) as w1ps, \
             tc.tile_pool(name="w1pb", bufs=5) as w1pb, \
             tc.tile_pool(name="w2p", bufs=2) as w2p, \
             tc.tile_pool(name="htp", bufs=KF) as htp, \
             tc.tile_pool(name="outp", bufs=3) as outp, \
             tc.tile_pool(name="psp", bufs=8, space="PSUM") as psp:

            # startup loads + PE warmup (keeps HAM at K=8/8 through the
            # initial DMA window; zeroed scratch, results never read)
            xts = {}
            for j in range(E_LOC):
                if j == 0:
                    t_ = consts.tile([P, KD * Cs[j]], dt, tag=f"xt{j}",
                                     name=f"xt{j}")
                    nc.scalar.dma_start(t_[:], xt_d[j][:, :])
                    xts[j] = t_
            b1_t = consts.tile([P, E_LOC * KF], f32, tag="b1")
            nc.scalar.dma_start(b1_t[:], b1[:, :])
            wsc = consts.tile([P, 512], dt, tag="wsc")
            nc.vector.memset(wsc[:], 0.0)
            wps = psp.tile([P, 512], f32, tag="ps")
            for _ in range(NWARM):
                nc.tensor.matmul(wps[:], wsc[:, :P], wsc[:],
                                 start=True, stop=True)

            w1ts = {}
            w2ts = {}
            prb_t = [None]
            cur = [0]

            def emit_until(tnow):
                while cur[0] < len(events) and events[cur[0]][0] <= tnow:
                    _, _, kind, e, i = events[cur[0]]
                    cur[0] += 1
                    if kind == 'w1':
                        cs, w = _w1_chunks(e)[i]
                        pool = w1ps if w == 256 else w1pb
                        t_ = pool.tile([P, KD * (256 if w == 256 else 1024)],
                                       dt, tag="w1", name="w1t")
                        nc.sync.dma_start(
                            t_[:, :KD * w],
                            w1[e, :, KD * cs:KD * (cs + w)])
                        w1ts[(e, i)] = t_
                    elif kind == 'w2':
                        t_ = w2p.tile([P, G * D_MODEL], dt, tag="w2",
                                      name="w2t")
                        nc.scalar.dma_start(t_[:], w2[e, i, :, :])
                        w2ts[(e, i)] = t_
                    elif kind == 'xt':
                        t_ = consts.tile([P, KD * Cs[e]], dt, tag=f"xt{e}",
                                         name=f"xt{e}")
                        nc.scalar.dma_start(t_[:], xt_d[e][:, :])
                        xts[e] = t_
                    else:
                        t_ = consts.tile([P, CT], f32, tag="prb", name="prb")
                        nc.scalar.dma_start(t_[:], prb[:, :])
                        prb_t[0] = t_

            mt = 0.0
            for _ in range(repeat):
                for e in range(E_LOC):
                    C = Cs[e]
                    chunks = _w1_chunks(e)
                    # phase A: ht[m] = gelu(w1.T @ x + b1)  [P dff x C tok]
                    hts = []
                    for m in range(KF):
                        emit_until(mt)
                        pa = psp.tile([P, 512], f32, tag="ps", name="pa")
                        col = m * P
                        ci = next(i for i, (cs, w) in enumerate(chunks)
                                  if cs <= col < cs + w)
                        cs, w = chunks[ci]
                        lc = (col - cs) // P
                        for k in range(KD):
                            nc.tensor.matmul(
                                pa[:, :C],
                                w1ts[(e, ci)][:, k * w + lc * P:
                                              k * w + (lc + 1) * P],
                                xts[e][:, k * C:k * C + C],
                                start=(k == 0), stop=(k == KD - 1))
                        ht = htp.tile([P, Cs[0]], dt, tag="ht", name="ht")
                        nc.scalar.activation(
                            ht[:, :C], pa[:, :C],
                            mybir.ActivationFunctionType.Gelu,
                            bias=b1_t[:, e * KF + m: e * KF + m + 1])
                        hts.append(ht)
                        mt += tA[e] / KF
                    # phase B (transposed): pb[md] = sum_k w2_k.T @ ht_k
                    pbs = [psp.tile([P, 512], f32, tag="ps", name="pb")
                           for _md in range(MD)]
                    for k in range(KF):
                        emit_until(mt)
                        kc, g = divmod(k, G)
                        for md in range(MD):
                            nc.tensor.matmul(
                                pbs[md][:, :C],
                                w2ts[(e, kc)][:, g * D_MODEL + md * P:
                                              g * D_MODEL + (md + 1) * P],
                                hts[k][:, :C],
                                start=(k == 0), stop=(k == KF - 1))
                        mt += tA[e] / KF
                    # drain: yT[md] = prob * pb[md]
                    for md in range(MD):
                        ot = outp.tile([P, 512], dt, tag="out", name="ot")
                        nc.vector.scalar_tensor_tensor(
                            ot[:, :C], pbs[md][:, :C], 1.0,
                            prb_t[0][:, offs[e]:offs[e] + C], mult, mult)
                        nc.scalar.dma_start(
                            y_d[e][md * P:(md + 1) * P, :], ot[:, :C])
    nc.finalize()
    return nc


def _route(x, gate_w, gate_b):
    """Top-2 routing on host. Returns flattened (expert, prob) per routed
    pair, the by-expert sort order, per-expert counts/starts, and each
    pair's position within its expert segment."""
    T = x.shape[0]
    scores = x @ gate_w + gate_b                      # [T, E]
    part = np.argpartition(scores, -TOP_K, axis=1)[:, -TOP_K:]   # [T, 2]
    vals = np.take_along_axis(scores, part, axis=1)
    vmax = vals.max(axis=1, keepdims=True)
    ex = np.exp(vals - vmax)
    prob = ex / ex.sum(axis=1, keepdims=True)

    expert_flat = part.ravel()                        # [2T]
    prob_flat = prob.ravel().astype(np.float32)
    token_flat = np.repeat(np.arange(T), TOP_K)

    order = np.argsort(expert_flat, kind="stable")
    counts = np.bincount(expert_flat, minlength=NUM_EXPERTS)
    starts = np.zeros(NUM_EXPERTS + 1, dtype=np.int64)
    np.cumsum(counts, out=starts[1:])

    inv_order = np.empty_like(order)
    inv_order[order] = np.arange(order.size)
    pos = inv_order - starts[expert_flat]
    return (expert_flat, prob_flat, token_flat, order, counts, starts, pos)


def _prepare(x, gate_w, gate_b, w1, b1, w2, b2):
    """Host-side routing, balanced expert->(core,slot) assignment, and
    per-core input packing. Returns (in_maps, Cs, meta-for-combine)."""
    B, S, D = x.shape
    T = B * S
    xf = np.ascontiguousarray(x.reshape(T, D), dtype=np.float32)

    (expert_flat, prob_flat, token_flat, order, counts, starts, pos) = _route(
        xf, np.asarray(gate_w, np.float32), np.asarray(gate_b, np.float32))

    # balanced assignment: slot j of core c holds expert_desc[j*8 + c]
    expert_desc = np.argsort(-counts, kind="stable")
    core_of = np.empty(NUM_EXPERTS, dtype=np.int64)
    slot_of = np.empty(NUM_EXPERTS, dtype=np.int64)
    for j in range(E_LOC):
        for c in range(N_CORES):
            e = expert_desc[j * N_CORES + c]
            core_of[e] = c
            slot_of[e] = j
    Cs = []
    for j in range(E_LOC):
        mx = int(counts[expert_desc[j * N_CORES:(j + 1) * N_CORES]].max())
        Cs.append(max(16, -(-mx // 16) * 16))        # 16-aligned exact cap
        assert Cs[j] <= 512
    CT = sum(Cs)
    offs = [sum(Cs[:j]) for j in range(E_LOC)]

    xg16 = xf[token_flat[order]].astype(BF16)         # [2T, D] sorted by expert
    sorted_probs = prob_flat[order]

    w1_16 = np.asarray(w1, np.float32).astype(BF16)   # [E, D, F]
    w2_16 = np.asarray(w2, np.float32).astype(BF16)   # [E, F, D]
    b1_f = np.asarray(b1, np.float32)                 # [E, F]

    in_maps = []
    for c in range(N_CORES):
        m = {}
        w1_core = np.empty((E_LOC, P, KD * D_FF), dtype=BF16)
        w2_core = np.empty((E_LOC, KF // G, P, G * D_MODEL), dtype=BF16)
        b1_core = np.empty((E_LOC, D_FF), dtype=np.float32)
        prb_core = np.zeros((P, CT), dtype=np.float32)
        for j in range(E_LOC):
            e = expert_desc[j * N_CORES + c]
            c_e = int(counts[e])
            xt_j = np.zeros((D, Cs[j]), dtype=BF16)
            if c_e:
                seg = slice(starts[e], starts[e] + c_e)
                xt_j[:, :c_e] = xg16[seg].T
                prb_core[:, offs[j]:offs[j] + c_e] = sorted_probs[seg][None, :]
            # xt packed [P, KD*C]: [p, k*C+c] = x[d=k*128+p, tok c]
            m[f"xt{j}"] = np.ascontiguousarray(
                xt_j.reshape(KD, P, Cs[j]).transpose(1, 0, 2)
                .reshape(P, KD * Cs[j]))
            # w1 packed per chunk: [p, (chunk-major) k, col] blocks
            w1r = w1_16[e].reshape(KD, P, D_FF)
            blocks = [w1r[:, :, cs:cs + w].transpose(1, 0, 2)
                      .reshape(P, KD * w) for cs, w in _w1_chunks(j)]
            w1_core[j] = np.concatenate(blocks, axis=1)
            # w2 packed per G-chunk: [p, g*D + d] = w2[f=(kc*G+g)*128+p, d]
            w2r = w2_16[e].reshape(KF // G, G, P, D_MODEL)
            w2_core[j] = w2r.transpose(0, 2, 1, 3).reshape(
                KF // G, P, G * D_MODEL)
            b1_core[j] = b1_f[e]
        m["w1"] = np.ascontiguousarray(w1_core)
        m["w2"] = np.ascontiguousarray(w2_core)
        m["prb"] = np.ascontiguousarray(prb_core)
        m["b1"] = np.ascontiguousarray(
            b1_core.reshape(E_LOC, KF, P).transpose(2, 0, 1)
            .reshape(P, E_LOC * KF))
        in_maps.append(m)

    meta = dict(T=T, shape=x.shape, CT=CT, offs=offs,
                core_of=core_of, slot_of=slot_of,
                expert_flat=expert_flat, prob_flat=prob_flat,
                token_flat=token_flat, pos=pos, b2=np.asarray(b2, np.float32))
    return in_maps, Cs, meta


def _combine(y_per_core, meta):
    """out[t] = sum of the token's two routed expert outputs (+ b2 term).
    Each per-core entry is [y0, y1, y2] with yj = [D_MODEL, C_j]."""
    T = meta["T"]
    CT = meta["CT"]
    offs = np.asarray(meta["offs"], dtype=np.int64)
    expert_flat = meta["expert_flat"]
    yt = np.concatenate(
        [np.concatenate(ys, axis=1) for ys in y_per_core],
        axis=1).astype(np.float32)                     # [D, 8*CT]

    cols = (meta["core_of"][expert_flat] * CT
            + offs[meta["slot_of"][expert_flat]] + meta["pos"])
    cols = cols.reshape(T, TOP_K)
    out = (yt[:, cols[:, 0]] + yt[:, cols[:, 1]]).T    # [T, D]

    b2_f = meta["b2"]
    if np.any(b2_f):
        combine = np.zeros((T, NUM_EXPERTS), dtype=np.float32)
        np.add.at(combine, (meta["token_flat"], expert_flat), meta["prob_flat"])
        out = out + combine @ b2_f
    return np.ascontiguousarray(out.reshape(meta["shape"]), dtype=np.float32)


def kernel(x, gate_w, gate_b, w1, b1, w2, b2):
    from concourse import bass_utils

    in_maps, Cs, meta = _prepare(x, gate_w, gate_b, w1, b1, w2, b2)
    nc = _build(Cs)
    res = bass_utils.run_bass_kernel_spmd(nc, in_maps, core_ids=list(range(N_CORES)))
    return _combine([[res.results[c][f"y{j}"] for j in range(E_LOC)]
                     for c in range(N_CORES)], meta)


# revision 13
# speedup vs baseline: 1.2645x; 1.0022x over previous
"""MoE layer (top-2 of 24 experts, d_model=1024, d_ff=4096, T=4096 tokens)
on 8 Trainium2 NeuronCores.

Strategy (expert-parallel, host-routed):
  - Host computes the gate, top-2 ids and softmax probs, gathers each
    expert's tokens into a transposed, k-packed buffer per slot.
  - Experts sharded 3 per core, balanced by token count (sorted-deal into
    3 slots); slot capacity = max count in that octile, 16-aligned exact.
  - Per expert on device (all matmul free dims = exact token count C):
      phase A: ht[m] = gelu(w1_km.T @ xT + b1)    32 m-groups, N=C
      phase B (transposed): 8 PSUM-resident banks pb[md] = yT d-chunks,
               k-outer sweep so w2 streams HBM exactly once:
               pb[md] += w2_k[:, md].T @ ht[k]    N=C, no 128-padding
      drain:   yT[md] = pb[md] * prob_broadcast   (DVE), DMA out
  - DMAs are k-packed into 0.5-2 MB transfers (deep intra-DMA pipelining)
    and emitted in deadline-sorted order on two HWDGE rings (sync: w1,
    scalar: x/w2/y); pool buffer recycling provides runtime pacing.
  - ~28 warmup matmuls on a zeroed scratch tile keep the PE HAM clock
    un-throttled (K=8/8) through the startup DMA window.
  - Host scatters the two per-token expert outputs back together.

Matmuls in bf16 with fp32 PSUM accumulation (rel err ~3e-3); b1 applied
exactly as the ACT per-partition bias.
"""

import numpy as np
import ml_dtypes

P = 128
D_MODEL = 1024
D_FF = 4096
NUM_EXPERTS = 24
TOP_K = 2
N_CORES = 8
E_LOC = NUM_EXPERTS // N_CORES   # 3 experts per core
KD = D_MODEL // P                # 8  k-chunks over d_model
KF = D_FF // P                   # 32 k-chunks over d_ff
MD = D_MODEL // P                # 8  output d-chunks (phase B)
G = 4                            # w2 k-tiles packed per DMA
BF16 = ml_dtypes.bfloat16
NWARM = 16                       # PE warmup matmuls


def _w1_chunks(e):
    """(col_start, width) chunks of w1's 4096 columns for expert slot e.
    First-processed expert uses small leading chunks for a fast start."""
    if e == 0:
        return [(0, 256), (256, 256), (512, 256), (768, 256),
                (1024, 1024), (2048, 1024), (3072, 1024)]
    return [(0, 1024), (1024, 1024), (2048, 1024), (3072, 1024)]


def _build(Cs, repeat=1):
    """Per-core Bass program (SPMD: same program, per-core data).

    Cs: per-slot token capacities (16-aligned, each <= 512).
    """
    import concourse.bacc as bacc
    import concourse.mybir as mybir
    from concourse.tile import TileContext

    dt = mybir.dt.bfloat16
    f32 = mybir.dt.float32
    CT = sum(Cs)
    offs = [sum(Cs[:j]) for j in range(E_LOC)]

    # model timeline (us) for DMA deadline sorting
    tA = [256.0 * C / 2400.0 for C in Cs]
    t0 = []
    t = 0.0
    for j in range(E_LOC):
        t0.append(t)
        t += 2.0 * tA[j]

    # (deadline, ring, kind, e, idx)  ring 0=sync(w1) 1=scalar(w2/xt/prb)
    events = []
    for e in range(E_LOC):
        for ci, (cs, w) in enumerate(_w1_chunks(e)):
            events.append((t0[e] + tA[e] * cs / D_FF - 12.0, 0, 'w1', e, ci))
        for kc in range(KF // G):
            events.append((t0[e] + tA[e] * (1.0 + kc * G / KF) - 8.0,
                           1, 'w2', e, kc))
        if e > 0:
            events.append((t0[e] - 20.0, 1, 'xt', e, 0))
    events.sort(key=lambda ev: (ev[0], ev[1]))

    nc = bacc.Bacc(None, target_bir_lowering=False)
    xt_d = [nc.dram_tensor(f"xt{j}", [P, KD * Cs[j]], dt, kind="ExternalInput")
            for j in range(E_LOC)]
    w1 = nc.dram_tensor("w1", [E_LOC, P, KD * D_FF], dt, kind="ExternalInput")
    w2 = nc.dram_tensor("w2", [E_LOC, KF // G, P, G * D_MODEL], dt,
                        kind="ExternalInput")
    b1 = nc.dram_tensor("b1", [P, E_LOC * KF], f32, kind="ExternalInput")
    y_d = [nc.dram_tensor(f"y{j}", [D_MODEL, Cs[j]], dt,
                          kind="ExternalOutput") for j in range(E_LOC)]

    with TileContext(nc) as tc:
        with tc.tile_pool(name="consts", bufs=1) as consts, \
             tc.tile_pool(name="w1ps", bufs=4) as w1ps, \
             tc.tile_pool(name="w1pb", bufs=5) as w1pb, \
             tc.tile_pool(name="w2p", bufs=2) as w2p, \
             tc.tile_pool(name="htp", bufs=KF) as htp, \
             tc.tile_pool(name="outp", bufs=3) as outp, \
             tc.tile_pool(name="psp", bufs=8, space="PSUM") as psp:

            # startup loads + PE warmup (keeps HAM at K=8/8 through the
            # initial DMA window; zeroed scratch, results never read)
            xts = {}
            for j in range(E_LOC):
                if j == 0:
                    t_ = consts.tile([P, KD * Cs[j]], dt, tag=f"xt{j}",
                                     name=f"xt{j}")
                    nc.scalar.dma_start(t_[:], xt_d[j][:, :])
                    xts[j] = t_
            b1_t = consts.tile([P, E_LOC * KF], f32, tag="b1")
            nc.scalar.dma_start(b1_t[:], b1[:, :])
            wsc = consts.tile([P, 512], dt, tag="wsc")
            nc.vector.memset(wsc[:], 0.0)
            wps = psp.tile([P, 512], f32, tag="ps")
            for _ in range(NWARM):
                nc.tensor.matmul(wps[:], wsc[:, :P], wsc[:],
                                 start=True, stop=True)

            w1ts = {}
            w2ts = {}
            cur = [0]

            def emit_until(tnow):
                while cur[0] < len(events) and events[cur[0]][0] <= tnow:
                    _, _, kind, e, i = events[cur[0]]
                    cur[0] += 1
                    if kind == 'w1':
                        cs, w = _w1_chunks(e)[i]
                        pool = w1ps if w == 256 else w1pb
                        t_ = pool.tile([P, KD * (256 if w == 256 else 1024)],
                                       dt, tag="w1", name="w1t")
                        nc.sync.dma_start(
                            t_[:, :KD * w],
                            w1[e, :, KD * cs:KD * (cs + w)])
                        w1ts[(e, i)] = t_
                    elif kind == 'w2':
                        t_ = w2p.tile([P, G * D_MODEL], dt, tag="w2",
                                      name="w2t")
                        nc.scalar.dma_start(t_[:], w2[e, i, :, :])
                        w2ts[(e, i)] = t_
                    else:
                        t_ = consts.tile([P, KD * Cs[e]], dt, tag=f"xt{e}",
                                         name=f"xt{e}")
                        nc.scalar.dma_start(t_[:], xt_d[e][:, :])
                        xts[e] = t_

            mt = 0.0
            for _ in range(repeat):
                for e in range(E_LOC):
                    C = Cs[e]
                    chunks = _w1_chunks(e)
                    # phase A: ht[m] = gelu(w1.T @ x + b1)  [P dff x C tok]
                    hts = []
                    for m in range(KF):
                        emit_until(mt)
                        pa = psp.tile([P, 512], f32, tag="ps", name="pa")
                        col = m * P
                        ci = next(i for i, (cs, w) in enumerate(chunks)
                                  if cs <= col < cs + w)
                        cs, w = chunks[ci]
                        lc = (col - cs) // P
                        for k in range(KD):
                            nc.tensor.matmul(
                                pa[:, :C],
                                w1ts[(e, ci)][:, k * w + lc * P:
                                              k * w + (lc + 1) * P],
                                xts[e][:, k * C:k * C + C],
                                start=(k == 0), stop=(k == KD - 1))
                        ht = htp.tile([P, Cs[0]], dt, tag="ht", name="ht")
                        nc.scalar.activation(
                            ht[:, :C], pa[:, :C],
                            mybir.ActivationFunctionType.Gelu,
                            bias=b1_t[:, e * KF + m: e * KF + m + 1])
                        hts.append(ht)
                        mt += tA[e] / KF
                    # phase B (transposed): pb[md] = sum_k w2_k.T @ ht_k
                    pbs = [psp.tile([P, 512], f32, tag="ps", name="pb")
                           for _md in range(MD)]
                    for k in range(KF - G):
                        emit_until(mt)
                        kc, g = divmod(k, G)
                        for md in range(MD):
                            nc.tensor.matmul(
                                pbs[md][:, :C],
                                w2ts[(e, kc)][:, g * D_MODEL + md * P:
                                              g * D_MODEL + (md + 1) * P],
                                hts[k][:, :C],
                                start=(k == 0), stop=False)
                        mt += tA[e] / KF
                    # last w2 chunk md-outer: each bank finishes 4 MMs apart
                    # so its drain copy (ACT/DVE alternating; prob scaling
                    # happens on host) pipelines into B's tail
                    emit_until(mt)
                    kc = KF // G - 1
                    for md in range(MD):
                        for g in range(G):
                            nc.tensor.matmul(
                                pbs[md][:, :C],
                                w2ts[(e, kc)][:, g * D_MODEL + md * P:
                                              g * D_MODEL + (md + 1) * P],
                                hts[kc * G + g][:, :C],
                                start=False, stop=(g == G - 1))
                        ot = outp.tile([P, 512], dt, tag="out", name="ot")
                        if md % 2 == 0:
                            nc.scalar.copy(ot[:, :C], pbs[md][:, :C])
                        else:
                            nc.vector.tensor_scalar_mul(
                                ot[:, :C], pbs[md][:, :C], 1.0)
                        nc.scalar.dma_start(
                            y_d[e][md * P:(md + 1) * P, :], ot[:, :C])
                    mt += G * tA[e] / KF
    nc.finalize()
    return nc


def _route(x, gate_w, gate_b):
    """Top-2 routing on host. Returns flattened (expert, prob) per routed
    pair, the by-expert sort order, per-expert counts/starts, and each
    pair's position within its expert segment."""
    T = x.shape[0]
    scores = x @ gate_w + gate_b                      # [T, E]
    part = np.argpartition(scores, -TOP_K, axis=1)[:, -TOP_K:]   # [T, 2]
    vals = np.take_along_axis(scores, part, axis=1)
    vmax = vals.max(axis=1, keepdims=True)
    ex = np.exp(vals - vmax)
    prob = ex / ex.sum(axis=1, keepdims=True)

    expert_flat = part.ravel()                        # [2T]
    prob_flat = prob.ravel().astype(np.float32)
    token_flat = np.repeat(np.arange(T), TOP_K)

    order = np.argsort(expert_flat, kind="stable")
    counts = np.bincount(expert_flat, minlength=NUM_EXPERTS)
    starts = np.zeros(NUM_EXPERTS + 1, dtype=np.int64)
    np.cumsum(counts, out=starts[1:])

    inv_order = np.empty_like(order)
    inv_order[order] = np.arange(order.size)
    pos = inv_order - starts[expert_flat]
    return (expert_flat, prob_flat, token_flat, order, counts, starts, pos)


def _prepare(x, gate_w, gate_b, w1, b1, w2, b2):
    """Host-side routing, balanced expert->(core,slot) assignment, and
    per-core input packing. Returns (in_maps, Cs, meta-for-combine)."""
    B, S, D = x.shape
    T = B * S
    xf = np.ascontiguousarray(x.reshape(T, D), dtype=np.float32)

    (expert_flat, prob_flat, token_flat, order, counts, starts, pos) = _route(
        xf, np.asarray(gate_w, np.float32), np.asarray(gate_b, np.float32))

    # balanced assignment: slot j of core c holds expert_desc[j*8 + c]
    expert_desc = np.argsort(-counts, kind="stable")
    core_of = np.empty(NUM_EXPERTS, dtype=np.int64)
    slot_of = np.empty(NUM_EXPERTS, dtype=np.int64)
    for j in range(E_LOC):
        for c in range(N_CORES):
            e = expert_desc[j * N_CORES + c]
            core_of[e] = c
            slot_of[e] = j
    Cs = []
    for j in range(E_LOC):
        mx = int(counts[expert_desc[j * N_CORES:(j + 1) * N_CORES]].max())
        Cs.append(max(16, -(-mx // 16) * 16))        # 16-aligned exact cap
        assert Cs[j] <= 512
    CT = sum(Cs)
    offs = [sum(Cs[:j]) for j in range(E_LOC)]

    xg16 = xf[token_flat[order]].astype(BF16)         # [2T, D] sorted by expert
    sorted_probs = prob_flat[order]

    w1_16 = np.asarray(w1, np.float32).astype(BF16)   # [E, D, F]
    w2_16 = np.asarray(w2, np.float32).astype(BF16)   # [E, F, D]
    b1_f = np.asarray(b1, np.float32)                 # [E, F]

    in_maps = []
    for c in range(N_CORES):
        m = {}
        w1_core = np.empty((E_LOC, P, KD * D_FF), dtype=BF16)
        w2_core = np.empty((E_LOC, KF // G, P, G * D_MODEL), dtype=BF16)
        b1_core = np.empty((E_LOC, D_FF), dtype=np.float32)
        for j in range(E_LOC):
            e = expert_desc[j * N_CORES + c]
            c_e = int(counts[e])
            xt_j = np.zeros((D, Cs[j]), dtype=BF16)
            if c_e:
                seg = slice(starts[e], starts[e] + c_e)
                xt_j[:, :c_e] = xg16[seg].T
            # xt packed [P, KD*C]: [p, k*C+c] = x[d=k*128+p, tok c]
            m[f"xt{j}"] = np.ascontiguousarray(
                xt_j.reshape(KD, P, Cs[j]).transpose(1, 0, 2)
                .reshape(P, KD * Cs[j]))
            # w1 packed per chunk: [p, (chunk-major) k, col] blocks
            w1r = w1_16[e].reshape(KD, P, D_FF)
            blocks = [w1r[:, :, cs:cs + w].transpose(1, 0, 2)
                      .reshape(P, KD * w) for cs, w in _w1_chunks(j)]
            w1_core[j] = np.concatenate(blocks, axis=1)
            # w2 packed per G-chunk: [p, g*D + d] = w2[f=(kc*G+g)*128+p, d]
            w2r = w2_16[e].reshape(KF // G, G, P, D_MODEL)
            w2_core[j] = w2r.transpose(0, 2, 1, 3).reshape(
                KF // G, P, G * D_MODEL)
            b1_core[j] = b1_f[e]
        m["w1"] = np.ascontiguousarray(w1_core)
        m["w2"] = np.ascontiguousarray(w2_core)
        m["b1"] = np.ascontiguousarray(
            b1_core.reshape(E_LOC, KF, P).transpose(2, 0, 1)
            .reshape(P, E_LOC * KF))
        in_maps.append(m)

    prv = np.zeros((CT * N_CORES,), dtype=np.float32)
    for c in range(N_CORES):
        for j in range(E_LOC):
            e = expert_desc[j * N_CORES + c]
            c_e = int(counts[e])
            if c_e:
                seg = slice(starts[e], starts[e] + c_e)
                base = c * CT + offs[j]
                prv[base:base + c_e] = sorted_probs[seg]
    meta = dict(T=T, shape=x.shape, CT=CT, offs=offs, prv=prv,
                core_of=core_of, slot_of=slot_of,
                expert_flat=expert_flat, prob_flat=prob_flat,
                token_flat=token_flat, pos=pos, b2=np.asarray(b2, np.float32))
    return in_maps, Cs, meta


def _combine(y_per_core, meta):
    """out[t] = sum of the token's two routed expert outputs (+ b2 term).
    Each per-core entry is [y0, y1, y2] with yj = [D_MODEL, C_j]."""
    T = meta["T"]
    CT = meta["CT"]
    offs = np.asarray(meta["offs"], dtype=np.int64)
    expert_flat = meta["expert_flat"]
    yt = np.concatenate(
        [np.concatenate(ys, axis=1) for ys in y_per_core],
        axis=1).astype(np.float32)                     # [D, 8*CT]

    cols = (meta["core_of"][expert_flat] * CT
            + offs[meta["slot_of"][expert_flat]] + meta["pos"])
    cols = cols.reshape(T, TOP_K)
    prv = meta["prv"]
    out = (yt[:, cols[:, 0]] * prv[cols[:, 0]]
           + yt[:, cols[:, 1]] * prv[cols[:, 1]]).T    # [T, D]

    b2_f = meta["b2"]
    if np.any(b2_f):
        combine = np.zeros((T, NUM_EXPERTS), dtype=np.float32)
        np.add.at(combine, (meta["token_flat"], expert_flat), meta["prob_flat"])
        out = out + combine @ b2_f
    return np.ascontiguousarray(out.reshape(meta["shape"]), dtype=np.float32)


def kernel(x, gate_w, gate_b, w1, b1, w2, b2):
    from concourse import bass_utils

    in_maps, Cs, meta = _prepare(x, gate_w, gate_b, w1, b1, w2, b2)
    nc = _build(Cs)
    res = bass_utils.run_bass_kernel_spmd(nc, in_maps, core_ids=list(range(N_CORES)))
    return _combine([[res.results[c][f"y{j}"] for j in range(E_LOC)]
                     for c in range(N_CORES)], meta)


# revision 14
# speedup vs baseline: 1.3086x; 1.0349x over previous
"""MoE layer (top-2 of 24 experts, d_model=1024, d_ff=4096, T=4096 tokens)
on 8 Trainium2 NeuronCores.

Strategy (expert-parallel, host-routed):
  - Host computes the gate, top-2 ids and softmax probs, gathers each
    expert's tokens into a transposed, k-packed buffer per slot.
  - Experts sharded 3 per core, balanced by token count (sorted-deal into
    3 slots); slot capacity = max count in that octile, 16-aligned exact.
  - Per expert on device (all matmul free dims = exact token count C):
      phase A: ht[m] = gelu(w1_km.T @ xT + b1)    32 m-groups, N=C
      phase B (transposed): 8 PSUM-resident banks pb[md] = yT d-chunks,
               k-outer sweep so w2 streams HBM exactly once:
               pb[md] += w2_k[:, md].T @ ht[k]    N=C, no 128-padding
      drain:   yT[md] = pb[md] * prob_broadcast   (DVE), DMA out
  - DMAs are k-packed into 0.5-2 MB transfers (deep intra-DMA pipelining)
    and emitted in deadline-sorted order on two HWDGE rings (sync: w1,
    scalar: x/w2/y); pool buffer recycling provides runtime pacing.
  - ~28 warmup matmuls on a zeroed scratch tile keep the PE HAM clock
    un-throttled (K=8/8) through the startup DMA window.
  - Host scatters the two per-token expert outputs back together.

Matmuls in bf16 with fp32 PSUM accumulation (rel err ~3e-3); b1 applied
exactly as the ACT per-partition bias.
"""

import numpy as np
import ml_dtypes

P = 128
D_MODEL = 1024
D_FF = 4096
NUM_EXPERTS = 24
TOP_K = 2
N_CORES = 8
E_LOC = NUM_EXPERTS // N_CORES   # 3 experts per core
KD = D_MODEL // P                # 8  k-chunks over d_model
KF = D_FF // P                   # 32 k-chunks over d_ff
MD = D_MODEL // P                # 8  output d-chunks (phase B)
G = 4                            # w2 k-tiles packed per DMA
BF16 = ml_dtypes.bfloat16
NWARM = 16                       # PE warmup matmuls


def _w1_chunks(e):
    """(col_start, width) chunks of w1's 4096 columns for expert slot e.
    First-processed expert uses small leading chunks for a fast start."""
    if e == 0:
        return [(0, 256), (256, 256), (512, 256), (768, 256),
                (1024, 1024), (2048, 1024), (3072, 1024)]
    return [(0, 1024), (1024, 1024), (2048, 1024), (3072, 1024)]


def _build(Cs, repeat=1):
    """Per-core Bass program (SPMD: same program, per-core data).

    Cs: per-slot token capacities (16-aligned, each <= 512).
    """
    import concourse.bacc as bacc
    import concourse.mybir as mybir
    from concourse.tile import TileContext

    dt = mybir.dt.bfloat16
    f32 = mybir.dt.float32
    CT = sum(Cs)
    offs = [sum(Cs[:j]) for j in range(E_LOC)]

    # model timeline (us) for DMA deadline sorting
    tA = [256.0 * C / 2400.0 for C in Cs]
    t0 = []
    t = 0.0
    for j in range(E_LOC):
        t0.append(t)
        t += 2.0 * tA[j]

    # (deadline, ring, kind, e, idx)  ring 0=sync(w1) 1=scalar(w2/xt/prb)
    events = []
    for e in range(E_LOC):
        for ci, (cs, w) in enumerate(_w1_chunks(e)):
            events.append((t0[e] + tA[e] * cs / D_FF - 12.0, 0, 'w1', e, ci))
        for kc in range(KF // G):
            events.append((t0[e] + tA[e] * (1.0 + kc * G / KF) - 10.0,
                           1, 'w2', e, kc))
        if e > 0:
            events.append((t0[e] - 20.0, 1, 'xt', e, 0))
    events.sort(key=lambda ev: (ev[0], ev[1]))

    nc = bacc.Bacc(None, target_bir_lowering=False)
    xt_d = [nc.dram_tensor(f"xt{j}", [P, KD * Cs[j]], dt, kind="ExternalInput")
            for j in range(E_LOC)]
    w1 = nc.dram_tensor("w1", [E_LOC, P, KD * D_FF], dt, kind="ExternalInput")
    w2 = nc.dram_tensor("w2", [E_LOC, KF // G, P, G * D_MODEL], dt,
                        kind="ExternalInput")
    b1 = nc.dram_tensor("b1", [P, E_LOC * KF], f32, kind="ExternalInput")
    y_d = [nc.dram_tensor(f"y{j}", [D_MODEL, Cs[j]], dt,
                          kind="ExternalOutput") for j in range(E_LOC)]

    with TileContext(nc) as tc:
        with tc.tile_pool(name="consts", bufs=1) as consts, \
             tc.tile_pool(name="w1ps", bufs=4) as w1ps, \
             tc.tile_pool(name="w1pb", bufs=5) as w1pb, \
             tc.tile_pool(name="w2p", bufs=3) as w2p, \
             tc.tile_pool(name="htp", bufs=KF) as htp, \
             tc.tile_pool(name="outp", bufs=3) as outp, \
             tc.tile_pool(name="psp", bufs=8, space="PSUM") as psp:

            # startup loads + PE warmup (keeps HAM at K=8/8 through the
            # initial DMA window; zeroed scratch, results never read)
            xts = {}
            for j in range(E_LOC):
                if j == 0:
                    t_ = consts.tile([P, KD * Cs[j]], dt, tag=f"xt{j}",
                                     name=f"xt{j}")
                    nc.scalar.dma_start(t_[:], xt_d[j][:, :])
                    xts[j] = t_
            b1_t = consts.tile([P, E_LOC * KF], f32, tag="b1")
            nc.scalar.dma_start(b1_t[:], b1[:, :])
            wsc = consts.tile([P, 512], dt, tag="wsc")
            nc.vector.memset(wsc[:], 0.0)
            wps = psp.tile([P, 512], f32, tag="ps")
            for _ in range(NWARM):
                nc.tensor.matmul(wps[:], wsc[:, :P], wsc[:],
                                 start=True, stop=True)

            w1ts = {}
            w2ts = {}
            cur = [0]

            def emit_until(tnow):
                while cur[0] < len(events) and events[cur[0]][0] <= tnow:
                    _, _, kind, e, i = events[cur[0]]
                    cur[0] += 1
                    if kind == 'w1':
                        cs, w = _w1_chunks(e)[i]
                        pool = w1ps if w == 256 else w1pb
                        t_ = pool.tile([P, KD * (256 if w == 256 else 1024)],
                                       dt, tag="w1", name="w1t")
                        nc.sync.dma_start(
                            t_[:, :KD * w],
                            w1[e, :, KD * cs:KD * (cs + w)])
                        w1ts[(e, i)] = t_
                    elif kind == 'w2':
                        t_ = w2p.tile([P, G * D_MODEL], dt, tag="w2",
                                      name="w2t")
                        nc.scalar.dma_start(t_[:], w2[e, i, :, :])
                        w2ts[(e, i)] = t_
                    else:
                        t_ = consts.tile([P, KD * Cs[e]], dt, tag=f"xt{e}",
                                         name=f"xt{e}")
                        nc.scalar.dma_start(t_[:], xt_d[e][:, :])
                        xts[e] = t_

            mt = 0.0
            for _ in range(repeat):
                for e in range(E_LOC):
                    C = Cs[e]
                    chunks = _w1_chunks(e)
                    # phase A: ht[m] = gelu(w1.T @ x + b1)  [P dff x C tok]
                    hts = []
                    for m in range(KF):
                        emit_until(mt)
                        pa = psp.tile([P, 512], f32, tag="ps", name="pa")
                        col = m * P
                        ci = next(i for i, (cs, w) in enumerate(chunks)
                                  if cs <= col < cs + w)
                        cs, w = chunks[ci]
                        lc = (col - cs) // P
                        for k in range(KD):
                            nc.tensor.matmul(
                                pa[:, :C],
                                w1ts[(e, ci)][:, k * w + lc * P:
                                              k * w + (lc + 1) * P],
                                xts[e][:, k * C:k * C + C],
                                start=(k == 0), stop=(k == KD - 1))
                        ht = htp.tile([P, Cs[0]], dt, tag="ht", name="ht")
                        nc.scalar.activation(
                            ht[:, :C], pa[:, :C],
                            mybir.ActivationFunctionType.Gelu,
                            bias=b1_t[:, e * KF + m: e * KF + m + 1])
                        hts.append(ht)
                        mt += tA[e] / KF
                    # phase B (transposed): pb[md] = sum_k w2_k.T @ ht_k
                    pbs = [psp.tile([P, 512], f32, tag="ps", name="pb")
                           for _md in range(MD)]
                    for k in range(KF - G):
                        emit_until(mt)
                        kc, g = divmod(k, G)
                        for md in range(MD):
                            nc.tensor.matmul(
                                pbs[md][:, :C],
                                w2ts[(e, kc)][:, g * D_MODEL + md * P:
                                              g * D_MODEL + (md + 1) * P],
                                hts[k][:, :C],
                                start=(k == 0), stop=False)
                        mt += tA[e] / KF
                    # last w2 chunk md-outer: each bank finishes 4 MMs apart
                    # so its drain copy (ACT/DVE alternating; prob scaling
                    # happens on host) pipelines into B's tail
                    emit_until(mt)
                    kc = KF // G - 1
                    for md in range(MD):
                        for g in range(G):
                            nc.tensor.matmul(
                                pbs[md][:, :C],
                                w2ts[(e, kc)][:, g * D_MODEL + md * P:
                                              g * D_MODEL + (md + 1) * P],
                                hts[kc * G + g][:, :C],
                                start=False, stop=(g == G - 1))
                        ot = outp.tile([P, 512], dt, tag="out", name="ot")
                        if md % 2 == 0:
                            nc.scalar.copy(ot[:, :C], pbs[md][:, :C])
                        else:
                            nc.vector.tensor_scalar_mul(
                                ot[:, :C], pbs[md][:, :C], 1.0)
                        nc.sync.dma_start(
                            y_d[e][md * P:(md + 1) * P, :], ot[:, :C])
                    mt += G * tA[e] / KF
    nc.finalize()
    return nc


def _route(x, gate_w, gate_b):
    """Top-2 routing on host. Returns flattened (expert, prob) per routed
    pair, the by-expert sort order, per-expert counts/starts, and each
    pair's position within its expert segment."""
    T = x.shape[0]
    scores = x @ gate_w + gate_b                      # [T, E]
    part = np.argpartition(scores, -TOP_K, axis=1)[:, -TOP_K:]   # [T, 2]
    vals = np.take_along_axis(scores, part, axis=1)
    vmax = vals.max(axis=1, keepdims=True)
    ex = np.exp(vals - vmax)
    prob = ex / ex.sum(axis=1, keepdims=True)

    expert_flat = part.ravel()                        # [2T]
    prob_flat = prob.ravel().astype(np.float32)
    token_flat = np.repeat(np.arange(T), TOP_K)

    order = np.argsort(expert_flat, kind="stable")
    counts = np.bincount(expert_flat, minlength=NUM_EXPERTS)
    starts = np.zeros(NUM_EXPERTS + 1, dtype=np.int64)
    np.cumsum(counts, out=starts[1:])

    inv_order = np.empty_like(order)
    inv_order[order] = np.arange(order.size)
    pos = inv_order - starts[expert_flat]
    return (expert_flat, prob_flat, token_flat, order, counts, starts, pos)


def _prepare(x, gate_w, gate_b, w1, b1, w2, b2):
    """Host-side routing, balanced expert->(core,slot) assignment, and
    per-core input packing. Returns (in_maps, Cs, meta-for-combine)."""
    B, S, D = x.shape
    T = B * S
    xf = np.ascontiguousarray(x.reshape(T, D), dtype=np.float32)

    (expert_flat, prob_flat, token_flat, order, counts, starts, pos) = _route(
        xf, np.asarray(gate_w, np.float32), np.asarray(gate_b, np.float32))

    # balanced assignment: slot j of core c holds expert_desc[j*8 + c]
    expert_desc = np.argsort(-counts, kind="stable")
    core_of = np.empty(NUM_EXPERTS, dtype=np.int64)
    slot_of = np.empty(NUM_EXPERTS, dtype=np.int64)
    for j in range(E_LOC):
        for c in range(N_CORES):
            e = expert_desc[j * N_CORES + c]
            core_of[e] = c
            slot_of[e] = j
    Cs = []
    for j in range(E_LOC):
        mx = int(counts[expert_desc[j * N_CORES:(j + 1) * N_CORES]].max())
        Cs.append(max(16, -(-mx // 16) * 16))        # 16-aligned exact cap
        assert Cs[j] <= 512
    CT = sum(Cs)
    offs = [sum(Cs[:j]) for j in range(E_LOC)]

    xg16 = xf[token_flat[order]].astype(BF16)         # [2T, D] sorted by expert
    sorted_probs = prob_flat[order]

    w1_16 = np.asarray(w1, np.float32).astype(BF16)   # [E, D, F]
    w2_16 = np.asarray(w2, np.float32).astype(BF16)   # [E, F, D]
    b1_f = np.asarray(b1, np.float32)                 # [E, F]

    in_maps = []
    for c in range(N_CORES):
        m = {}
        w1_core = np.empty((E_LOC, P, KD * D_FF), dtype=BF16)
        w2_core = np.empty((E_LOC, KF // G, P, G * D_MODEL), dtype=BF16)
        b1_core = np.empty((E_LOC, D_FF), dtype=np.float32)
        for j in range(E_LOC):
            e = expert_desc[j * N_CORES + c]
            c_e = int(counts[e])
            xt_j = np.zeros((D, Cs[j]), dtype=BF16)
            if c_e:
                seg = slice(starts[e], starts[e] + c_e)
                xt_j[:, :c_e] = xg16[seg].T
            # xt packed [P, KD*C]: [p, k*C+c] = x[d=k*128+p, tok c]
            m[f"xt{j}"] = np.ascontiguousarray(
                xt_j.reshape(KD, P, Cs[j]).transpose(1, 0, 2)
                .reshape(P, KD * Cs[j]))
            # w1 packed per chunk: [p, (chunk-major) k, col] blocks
            w1r = w1_16[e].reshape(KD, P, D_FF)
            blocks = [w1r[:, :, cs:cs + w].transpose(1, 0, 2)
                      .reshape(P, KD * w) for cs, w in _w1_chunks(j)]
            w1_core[j] = np.concatenate(blocks, axis=1)
            # w2 packed per G-chunk: [p, g*D + d] = w2[f=(kc*G+g)*128+p, d]
            w2r = w2_16[e].reshape(KF // G, G, P, D_MODEL)
            w2_core[j] = w2r.transpose(0, 2, 1, 3).reshape(
                KF // G, P, G * D_MODEL)
            b1_core[j] = b1_f[e]
        m["w1"] = np.ascontiguousarray(w1_core)
        m["w2"] = np.ascontiguousarray(w2_core)
        m["b1"] = np.ascontiguousarray(
            b1_core.reshape(E_LOC, KF, P).transpose(2, 0, 1)
            .reshape(P, E_LOC * KF))
        in_maps.append(m)

    prv = np.zeros((CT * N_CORES,), dtype=np.float32)
    for c in range(N_CORES):
        for j in range(E_LOC):
            e = expert_desc[j * N_CORES + c]
            c_e = int(counts[e])
            if c_e:
                seg = slice(starts[e], starts[e] + c_e)
                base = c * CT + offs[j]
                prv[base:base + c_e] = sorted_probs[seg]
    meta = dict(T=T, shape=x.shape, CT=CT, offs=offs, prv=prv,
                core_of=core_of, slot_of=slot_of,
                expert_flat=expert_flat, prob_flat=prob_flat,
                token_flat=token_flat, pos=pos, b2=np.asarray(b2, np.float32))
    return in_maps, Cs, meta


def _combine(y_per_core, meta):
    """out[t] = sum of the token's two routed expert outputs (+ b2 term).
    Each per-core entry is [y0, y1, y2] with yj = [D_MODEL, C_j]."""
    T = meta["T"]
    CT = meta["CT"]
    offs = np.asarray(meta["offs"], dtype=np.int64)
    expert_flat = meta["expert_flat"]
    yt = np.concatenate(
        [np.concatenate(ys, axis=1) for ys in y_per_core],
        axis=1).astype(np.float32)                     # [D, 8*CT]

    cols = (meta["core_of"][expert_flat] * CT
            + offs[meta["slot_of"][expert_flat]] + meta["pos"])
    cols = cols.reshape(T, TOP_K)
    prv = meta["prv"]
    out = (yt[:, cols[:, 0]] * prv[cols[:, 0]]
           + yt[:, cols[:, 1]] * prv[cols[:, 1]]).T    # [T, D]

    b2_f = meta["b2"]
    if np.any(b2_f):
        combine = np.zeros((T, NUM_EXPERTS), dtype=np.float32)
        np.add.at(combine, (meta["token_flat"], expert_flat), meta["prob_flat"])
        out = out + combine @ b2_f
    return np.ascontiguousarray(out.reshape(meta["shape"]), dtype=np.float32)


def kernel(x, gate_w, gate_b, w1, b1, w2, b2):
    from concourse import bass_utils

    in_maps, Cs, meta = _prepare(x, gate_w, gate_b, w1, b1, w2, b2)
    nc = _build(Cs)
    res = bass_utils.run_bass_kernel_spmd(nc, in_maps, core_ids=list(range(N_CORES)))
    return _combine([[res.results[c][f"y{j}"] for j in range(E_LOC)]
                     for c in range(N_CORES)], meta)


# revision 15
# speedup vs baseline: 1.3385x; 1.0228x over previous
"""MoE layer (top-2 of 24 experts, d_model=1024, d_ff=4096, T=4096 tokens)
on 8 Trainium2 NeuronCores.

Strategy (expert-parallel, host-routed):
  - Host computes the gate, top-2 ids and softmax probs, gathers each
    expert's tokens into a transposed, k-packed buffer per slot.
  - Experts sharded 3 per core, balanced by token count (sorted-deal into
    3 slots); slot capacity = max count in that octile, 16-aligned exact.
  - Per expert on device (all matmul free dims = exact token count C):
      phase A: ht[m] = gelu(w1_km.T @ xT + b1)    32 m-groups, N=C
      phase B (transposed): 8 PSUM-resident banks pb[md] = yT d-chunks,
               k-outer sweep so w2 streams HBM exactly once:
               pb[md] += w2_k[:, md].T @ ht[k]    N=C, no 128-padding
      drain:   yT[md] = pb[md] * prob_broadcast   (DVE), DMA out
  - DMAs are k-packed into 0.5-2 MB transfers (deep intra-DMA pipelining)
    and emitted in deadline-sorted order on two HWDGE rings (sync: w1,
    scalar: x/w2/y); pool buffer recycling provides runtime pacing.
  - ~28 warmup matmuls on a zeroed scratch tile keep the PE HAM clock
    un-throttled (K=8/8) through the startup DMA window.
  - Host scatters the two per-token expert outputs back together.

Matmuls in bf16 with fp32 PSUM accumulation (rel err ~3e-3); b1 applied
exactly as the ACT per-partition bias.
"""

import numpy as np
import ml_dtypes

P = 128
D_MODEL = 1024
D_FF = 4096
NUM_EXPERTS = 24
TOP_K = 2
N_CORES = 8
E_LOC = NUM_EXPERTS // N_CORES   # 3 experts per core
KD = D_MODEL // P                # 8  k-chunks over d_model
KF = D_FF // P                   # 32 k-chunks over d_ff
MD = D_MODEL // P                # 8  output d-chunks (phase B)
G = 4                            # w2 k-tiles packed per DMA
BF16 = ml_dtypes.bfloat16
NWARM = 16                       # PE warmup matmuls


def _w1_chunks(e):
    """(col_start, width) chunks of w1's 4096 columns for expert slot e.
    First-processed expert uses small leading chunks for a fast start."""
    if e == 0:
        return [(0, 256), (256, 256), (512, 256), (768, 256),
                (1024, 1024), (2048, 1024), (3072, 1024)]
    return [(0, 1024), (1024, 1024), (2048, 1024), (3072, 1024)]


def _build(Cs, repeat=1):
    """Per-core Bass program (SPMD: same program, per-core data).

    Cs: per-slot token capacities (16-aligned, each <= 512).
    """
    import concourse.bacc as bacc
    import concourse.mybir as mybir
    from concourse.tile import TileContext

    dt = mybir.dt.bfloat16
    f32 = mybir.dt.float32
    CT = sum(Cs)
    offs = [sum(Cs[:j]) for j in range(E_LOC)]

    # model timeline (us) for DMA deadline sorting
    tA = [256.0 * C / 2400.0 for C in Cs]
    t0 = []
    t = 0.0
    for j in range(E_LOC):
        t0.append(t)
        t += 2.0 * tA[j]

    # (deadline, ring, kind, e, idx)  ring 0=sync(w1) 1=scalar(w2/xt/prb)
    events = []
    for e in range(E_LOC):
        for ci, (cs, w) in enumerate(_w1_chunks(e)):
            ring = 1 if (e == 0 and ci in (4, 6)) else 0
            events.append((t0[e] + tA[e] * cs / D_FF - 12.0, ring,
                           'w1', e, ci))
        for kc in range(KF // G):
            events.append((t0[e] + tA[e] * (1.0 + kc * G / KF) - 10.0,
                           1, 'w2', e, kc))
        if e > 0:
            events.append((t0[e] - 20.0, 1, 'xt', e, 0))
    events.sort(key=lambda ev: (ev[0], ev[1]))

    nc = bacc.Bacc(None, target_bir_lowering=False)
    xt_d = [nc.dram_tensor(f"xt{j}", [P, KD * Cs[j]], dt, kind="ExternalInput")
            for j in range(E_LOC)]
    w1 = nc.dram_tensor("w1", [E_LOC, P, KD * D_FF], dt, kind="ExternalInput")
    w2 = nc.dram_tensor("w2", [E_LOC, KF // G, P, G * D_MODEL], dt,
                        kind="ExternalInput")
    b1 = nc.dram_tensor("b1", [P, E_LOC * KF], f32, kind="ExternalInput")
    y_d = [nc.dram_tensor(f"y{j}", [D_MODEL, Cs[j]], dt,
                          kind="ExternalOutput") for j in range(E_LOC)]

    with TileContext(nc) as tc:
        with tc.tile_pool(name="consts", bufs=1) as consts, \
             tc.tile_pool(name="w1ps", bufs=4) as w1ps, \
             tc.tile_pool(name="w1pb", bufs=5) as w1pb, \
             tc.tile_pool(name="w2p", bufs=3) as w2p, \
             tc.tile_pool(name="htp", bufs=KF) as htp, \
             tc.tile_pool(name="outp", bufs=3) as outp, \
             tc.tile_pool(name="psp", bufs=8, space="PSUM") as psp:

            # startup loads + PE warmup (keeps HAM at K=8/8 through the
            # initial DMA window; zeroed scratch, results never read)
            xts = {}
            for j in range(E_LOC):
                if j == 0:
                    t_ = consts.tile([P, KD * Cs[j]], dt, tag=f"xt{j}",
                                     name=f"xt{j}")
                    nc.scalar.dma_start(t_[:], xt_d[j][:, :])
                    xts[j] = t_
            b1_t = consts.tile([P, E_LOC * KF], f32, tag="b1")
            nc.scalar.dma_start(b1_t[:], b1[:, :])
            wsc = consts.tile([P, 512], dt, tag="wsc")
            nc.vector.memset(wsc[:], 0.0)
            wps = psp.tile([P, 512], f32, tag="ps")
            for _ in range(NWARM):
                nc.tensor.matmul(wps[:], wsc[:, :P], wsc[:],
                                 start=True, stop=True)

            w1ts = {}
            w2ts = {}
            cur = [0]

            def emit_until(tnow):
                while cur[0] < len(events) and events[cur[0]][0] <= tnow:
                    _, _, kind, e, i = events[cur[0]]
                    cur[0] += 1
                    if kind == 'w1':
                        cs, w = _w1_chunks(e)[i]
                        pool = w1ps if w == 256 else w1pb
                        t_ = pool.tile([P, KD * (256 if w == 256 else 1024)],
                                       dt, tag="w1", name="w1t")
                        eng = nc.scalar if (e == 0 and i in (4, 6)) else nc.sync
                        eng.dma_start(
                            t_[:, :KD * w],
                            w1[e, :, KD * cs:KD * (cs + w)])
                        w1ts[(e, i)] = t_
                    elif kind == 'w2':
                        t_ = w2p.tile([P, G * D_MODEL], dt, tag="w2",
                                      name="w2t")
                        nc.scalar.dma_start(t_[:], w2[e, i, :, :])
                        w2ts[(e, i)] = t_
                    else:
                        t_ = consts.tile([P, KD * Cs[e]], dt, tag=f"xt{e}",
                                         name=f"xt{e}")
                        nc.scalar.dma_start(t_[:], xt_d[e][:, :])
                        xts[e] = t_

            mt = 0.0
            for _ in range(repeat):
                for e in range(E_LOC):
                    C = Cs[e]
                    chunks = _w1_chunks(e)
                    # phase A: ht[m] = gelu(w1.T @ x + b1)  [P dff x C tok]
                    hts = []
                    for m in range(KF):
                        emit_until(mt)
                        pa = psp.tile([P, 512], f32, tag="ps", name="pa")
                        col = m * P
                        ci = next(i for i, (cs, w) in enumerate(chunks)
                                  if cs <= col < cs + w)
                        cs, w = chunks[ci]
                        lc = (col - cs) // P
                        for k in range(KD):
                            nc.tensor.matmul(
                                pa[:, :C],
                                w1ts[(e, ci)][:, k * w + lc * P:
                                              k * w + (lc + 1) * P],
                                xts[e][:, k * C:k * C + C],
                                start=(k == 0), stop=(k == KD - 1))
                        ht = htp.tile([P, Cs[0]], dt, tag="ht", name="ht")
                        nc.scalar.activation(
                            ht[:, :C], pa[:, :C],
                            mybir.ActivationFunctionType.Gelu,
                            bias=b1_t[:, e * KF + m: e * KF + m + 1])
                        hts.append(ht)
                        mt += tA[e] / KF
                    # phase B (transposed): pb[md] = sum_k w2_k.T @ ht_k
                    pbs = [psp.tile([P, 512], f32, tag="ps", name="pb")
                           for _md in range(MD)]
                    for k in range(KF - G):
                        emit_until(mt)
                        kc, g = divmod(k, G)
                        for md in range(MD):
                            nc.tensor.matmul(
                                pbs[md][:, :C],
                                w2ts[(e, kc)][:, g * D_MODEL + md * P:
                                              g * D_MODEL + (md + 1) * P],
                                hts[k][:, :C],
                                start=(k == 0), stop=False)
                        mt += tA[e] / KF
                    # last w2 chunk md-outer: each bank finishes 4 MMs apart
                    # so its drain copy (ACT/DVE alternating; prob scaling
                    # happens on host) pipelines into B's tail
                    emit_until(mt)
                    kc = KF // G - 1
                    for md in range(MD):
                        for g in range(G):
                            nc.tensor.matmul(
                                pbs[md][:, :C],
                                w2ts[(e, kc)][:, g * D_MODEL + md * P:
                                              g * D_MODEL + (md + 1) * P],
                                hts[kc * G + g][:, :C],
                                start=False, stop=(g == G - 1))
                        ot = outp.tile([P, 512], dt, tag="out", name="ot")
                        if md % 2 == 0:
                            nc.scalar.copy(ot[:, :C], pbs[md][:, :C])
                        else:
                            nc.vector.tensor_scalar_mul(
                                ot[:, :C], pbs[md][:, :C], 1.0)
                        nc.sync.dma_start(
                            y_d[e][md * P:(md + 1) * P, :], ot[:, :C])
                    mt += G * tA[e] / KF
    nc.finalize()
    return nc


def _route(x, gate_w, gate_b):
    """Top-2 routing on host. Returns flattened (expert, prob) per routed
    pair, the by-expert sort order, per-expert counts/starts, and each
    pair's position within its expert segment."""
    T = x.shape[0]
    scores = x @ gate_w + gate_b                      # [T, E]
    part = np.argpartition(scores, -TOP_K, axis=1)[:, -TOP_K:]   # [T, 2]
    vals = np.take_along_axis(scores, part, axis=1)
    vmax = vals.max(axis=1, keepdims=True)
    ex = np.exp(vals - vmax)
    prob = ex / ex.sum(axis=1, keepdims=True)

    expert_flat = part.ravel()                        # [2T]
    prob_flat = prob.ravel().astype(np.float32)
    token_flat = np.repeat(np.arange(T), TOP_K)

    order = np.argsort(expert_flat, kind="stable")
    counts = np.bincount(expert_flat, minlength=NUM_EXPERTS)
    starts = np.zeros(NUM_EXPERTS + 1, dtype=np.int64)
    np.cumsum(counts, out=starts[1:])

    inv_order = np.empty_like(order)
    inv_order[order] = np.arange(order.size)
    pos = inv_order - starts[expert_flat]
    return (expert_flat, prob_flat, token_flat, order, counts, starts, pos)


def _prepare(x, gate_w, gate_b, w1, b1, w2, b2):
    """Host-side routing, balanced expert->(core,slot) assignment, and
    per-core input packing. Returns (in_maps, Cs, meta-for-combine)."""
    B, S, D = x.shape
    T = B * S
    xf = np.ascontiguousarray(x.reshape(T, D), dtype=np.float32)

    (expert_flat, prob_flat, token_flat, order, counts, starts, pos) = _route(
        xf, np.asarray(gate_w, np.float32), np.asarray(gate_b, np.float32))

    # balanced assignment: slot j of core c holds expert_desc[j*8 + c]
    expert_desc = np.argsort(-counts, kind="stable")
    core_of = np.empty(NUM_EXPERTS, dtype=np.int64)
    slot_of = np.empty(NUM_EXPERTS, dtype=np.int64)
    for j in range(E_LOC):
        for c in range(N_CORES):
            e = expert_desc[j * N_CORES + c]
            core_of[e] = c
            slot_of[e] = j
    Cs = []
    for j in range(E_LOC):
        mx = int(counts[expert_desc[j * N_CORES:(j + 1) * N_CORES]].max())
        Cs.append(max(16, -(-mx // 16) * 16))        # 16-aligned exact cap
        assert Cs[j] <= 512
    CT = sum(Cs)
    offs = [sum(Cs[:j]) for j in range(E_LOC)]

    xg16 = xf[token_flat[order]].astype(BF16)         # [2T, D] sorted by expert
    sorted_probs = prob_flat[order]

    w1_16 = np.asarray(w1, np.float32).astype(BF16)   # [E, D, F]
    w2_16 = np.asarray(w2, np.float32).astype(BF16)   # [E, F, D]
    b1_f = np.asarray(b1, np.float32)                 # [E, F]

    in_maps = []
    for c in range(N_CORES):
        m = {}
        w1_core = np.empty((E_LOC, P, KD * D_FF), dtype=BF16)
        w2_core = np.empty((E_LOC, KF // G, P, G * D_MODEL), dtype=BF16)
        b1_core = np.empty((E_LOC, D_FF), dtype=np.float32)
        for j in range(E_LOC):
            e = expert_desc[j * N_CORES + c]
            c_e = int(counts[e])
            xt_j = np.zeros((D, Cs[j]), dtype=BF16)
            if c_e:
                seg = slice(starts[e], starts[e] + c_e)
                xt_j[:, :c_e] = xg16[seg].T
            # xt packed [P, KD*C]: [p, k*C+c] = x[d=k*128+p, tok c]
            m[f"xt{j}"] = np.ascontiguousarray(
                xt_j.reshape(KD, P, Cs[j]).transpose(1, 0, 2)
                .reshape(P, KD * Cs[j]))
            # w1 packed per chunk: [p, (chunk-major) k, col] blocks
            w1r = w1_16[e].reshape(KD, P, D_FF)
            blocks = [w1r[:, :, cs:cs + w].transpose(1, 0, 2)
                      .reshape(P, KD * w) for cs, w in _w1_chunks(j)]
            w1_core[j] = np.concatenate(blocks, axis=1)
            # w2 packed per G-chunk: [p, g*D + d] = w2[f=(kc*G+g)*128+p, d]
            w2r = w2_16[e].reshape(KF // G, G, P, D_MODEL)
            w2_core[j] = w2r.transpose(0, 2, 1, 3).reshape(
                KF // G, P, G * D_MODEL)
            b1_core[j] = b1_f[e]
        m["w1"] = np.ascontiguousarray(w1_core)
        m["w2"] = np.ascontiguousarray(w2_core)
        m["b1"] = np.ascontiguousarray(
            b1_core.reshape(E_LOC, KF, P).transpose(2, 0, 1)
            .reshape(P, E_LOC * KF))
        in_maps.append(m)

    prv = np.zeros((CT * N_CORES,), dtype=np.float32)
    for c in range(N_CORES):
        for j in range(E_LOC):
            e = expert_desc[j * N_CORES + c]
            c_e = int(counts[e])
            if c_e:
                seg = slice(starts[e], starts[e] + c_e)
                base = c * CT + offs[j]
                prv[base:base + c_e] = sorted_probs[seg]
    meta = dict(T=T, shape=x.shape, CT=CT, offs=offs, prv=prv,
                core_of=core_of, slot_of=slot_of,
                expert_flat=expert_flat, prob_flat=prob_flat,
                token_flat=token_flat, pos=pos, b2=np.asarray(b2, np.float32))
    return in_maps, Cs, meta


def _combine(y_per_core, meta):
    """out[t] = sum of the token's two routed expert outputs (+ b2 term).
    Each per-core entry is [y0, y1, y2] with yj = [D_MODEL, C_j]."""
    T = meta["T"]
    CT = meta["CT"]
    offs = np.asarray(meta["offs"], dtype=np.int64)
    expert_flat = meta["expert_flat"]
    yt = np.concatenate(
        [np.concatenate(ys, axis=1) for ys in y_per_core],
        axis=1).astype(np.float32)                     # [D, 8*CT]

    cols = (meta["core_of"][expert_flat] * CT
            + offs[meta["slot_of"][expert_flat]] + meta["pos"])
    cols = cols.reshape(T, TOP_K)
    prv = meta["prv"]
    out = (yt[:, cols[:, 0]] * prv[cols[:, 0]]
           + yt[:, cols[:, 1]] * prv[cols[:, 1]]).T    # [T, D]

    b2_f = meta["b2"]
    if np.any(b2_f):
        combine = np.zeros((T, NUM_EXPERTS), dtype=np.float32)
        np.add.at(combine, (meta["token_flat"], expert_flat), meta["prob_flat"])
        out = out + combine @ b2_f
    return np.ascontiguousarray(out.reshape(meta["shape"]), dtype=np.float32)


def kernel(x, gate_w, gate_b, w1, b1, w2, b2):
    from concourse import bass_utils

    in_maps, Cs, meta = _prepare(x, gate_w, gate_b, w1, b1, w2, b2)
    nc = _build(Cs)
    res = bass_utils.run_bass_kernel_spmd(nc, in_maps, core_ids=list(range(N_CORES)))
    return _combine([[res.results[c][f"y{j}"] for j in range(E_LOC)]
                     for c in range(N_CORES)], meta)
